# revision 27
# baseline (speedup 1.0000x reference)
"""Trainium2 Bass kernel for nn_BitLinear (LayerNorm -> 1.58-bit BitLinear).

Math notes
----------
Reference computes, per the module:
    xn    = LN(x) * ln_gamma + ln_beta            (eps = 1e-3)
    beta  = mean(|W|);  w_q = clip(round(W / (beta + 1e-5)), -1, 1)
    gamma = max(|xn|)   (global absmax)
    xq    = clip(xn * 128 / gamma, -128 + 1e-5, 128 - 1e-5)
    y     = (xq @ w_q) * (gamma * beta / 128)

The gamma factor cancels exactly: (xn*128/gamma) @ w_q * (gamma*beta/128)
== (xn @ w_q) * beta.  The clip only affects elements within relative
7.8e-8 of the global absmax, changing them by ~1e-7 relative -- far below
f32 matmul roundoff.  So the kernel computes y = (LN(x) @ w_q) * beta,
which is fully data-parallel over tokens (no collectives needed).

w_q is exactly ternary; with c = 0.5*(beta+1e-5):
    2*w_q = 2*1[W > c] - 2*1[W < -c]  =  sign(W - c) + sign(W + c)
(2x scale folded into the output scale beta/2; exact in bf16), so the
matmul runs at bf16 PE rate.  Rounding: bf16 cast of the normalized
activations + bf16 y output (upcast on host); ~0.2% combined, far
under tolerance.

Sharding: data-parallel over the 32768 tokens, 4096 per core; weight
replicated (each core redundantly computes beta/w_q from the full W --
cheaper than a collective).

Scheduling notes (v3) -- measured constraints this is built around:
- ~850 ns descriptor-generation (DIRECT2D) per dma_start, serialized
  per HWDGE ring; ~150 ns per 4 KiB descriptor per DMA queue
  (~27 GB/s/queue, 16 queues).  So: few large dma_starts, W split
  across both rings right behind x super 0.
- The tile scheduler dispatches ready instructions by DAG depth, not
  emission order.  Ternarize sits deep (W->|W|->beta->c->tern), so
  late supers' shallow stats/normalize would preempt it exactly when
  the MM stream starts consuming wq k-blocks.  Supers 3+ get explicit
  ordering edges AFTER the last ternarize op on their engine.
- Six supers are prefetched and fronted before the MM stream begins:
  their PE transposes fill the otherwise-idle W-load window (keeps the
  HAM clock warm) and shorten the steady-state PE stream.
- Matmuls are 512-wide (ISA: moving operand <= 512 elem/partition),
  ternarize is emitted per k chunk in MM consumption order
  (just-in-time), alternating DVE(cmp/cmp/sub) and ACT(sign/sign)+DVE.
"""

import numpy as np

B, S, D, U = 4, 8192, 1024, 1024
N_CORES = 8
TOK = (B * S) // N_CORES  # 4096 tokens per core
P = 128
KB = D // P               # 8 contraction blocks
NTILES = TOK // P         # 32 token tiles per core
SUPER = 2                 # token tiles per DMA transfer (1 MiB chunks)
PRE_SUPERS = 6            # supers fronted before the MM stream
TERN_DEP_FROM = 2         # supers >= this yield to ternarize
LN_EPS = 1e-3
EPS = 1e-5

_NC_CACHE = {}


def _build(apply_gamma: bool, apply_beta: bool):
    """Build the single-core Bass program (SPMD: same NEFF on all 8 cores)."""
    import concourse.bacc as bacc
    import concourse.mybir as mybir
    import concourse.tile as tile
    from concourse.bass import ts
    from concourse.masks import make_identity
    from concourse.tile_rust import add_dep_helper

    fp32 = mybir.dt.float32
    bf16 = mybir.dt.bfloat16
    AF = mybir.ActivationFunctionType
    OP = mybir.AluOpType
    AX = mybir.AxisListType

    nc = bacc.Bacc()
    x_h = nc.dram_tensor("x", [TOK, D], fp32, kind="ExternalInput")
    w_h = nc.dram_tensor("weight", [D, U], fp32, kind="ExternalInput")
    g_h = (
        nc.dram_tensor("ln_gamma", [D], fp32, kind="ExternalInput")
        if apply_gamma
        else None
    )
    lb_h = (
        nc.dram_tensor("ln_beta", [D], fp32, kind="ExternalInput")
        if apply_beta
        else None
    )
    y_h = nc.dram_tensor("y", [TOK, U], bf16, kind="ExternalOutput")

    with tile.TileContext(nc) as tc:
        with (
            tc.tile_pool(name="singles", bufs=1) as singles,
            tc.tile_pool(name="prep", bufs=4) as prep,
            tc.tile_pool(name="xin", bufs=4) as xin_pool,
            tc.tile_pool(name="xn", bufs=6) as xn_pool,
            tc.tile_pool(name="xt", bufs=2 * PRE_SUPERS + 3) as xt_pool,
            tc.tile_pool(name="yout", bufs=3) as y_pool,
            tc.tile_pool(name="stats", bufs=6) as stats_pool,
            tc.tile_pool(name="ps_t", bufs=2, space="PSUM") as ps_t_pool,
            tc.tile_pool(name="ps_y", bufs=6, space="PSUM") as ps_y_pool,
        ):
            # ---- x super-tile 0 prefetch first on the sync ring ----
            x_view = x_h[:, :].rearrange("(o p) d -> p o d", p=P)
            y_view = y_h[:, :].rearrange("(o p) u -> p o u", p=P)

            def issue_x(j):
                x_sb = xin_pool.tile([P, SUPER, D], fp32, name="x_sb")
                nc.sync.dma_start(
                    out=x_sb, in_=x_view[:, j * SUPER : (j + 1) * SUPER, :]
                )
                return x_sb

            # super 0 split: tile 0 ahead of W on sync (only 0.5 MiB delays
            # the W stream), tile 1 behind W on the scalar ring -- it is
            # needed ~2 us later than tile 0 by the first back-super.
            x_first = xin_pool.tile([P, SUPER, D], fp32, name="x_first")
            nc.sync.dma_start(out=x_first[:, 0, :], in_=x_view[:, 0, :])

            # ---- W load: ko-major chunks, one half per HWDGE ring ----
            w_view = w_h[:, :].rearrange("(ko ki) u -> ki ko u", ki=P)
            w_sb = singles.tile([P, KB, U], fp32)
            nc.sync.dma_start(out=w_sb[:, 0 : KB // 2, :],
                              in_=w_view[:, 0 : KB // 2, :])
            nc.scalar.dma_start(out=w_sb[:, KB // 2 : KB, :],
                                in_=w_view[:, KB // 2 : KB, :])
            nc.scalar.dma_start(out=x_first[:, 1, :], in_=x_view[:, 1, :])

            if apply_gamma:
                g_sb = singles.tile([P, KB], fp32)
                nc.scalar.dma_start(
                    out=g_sb, in_=g_h[:].rearrange("(ko ki) -> ki ko", ki=P)
                )
            if apply_beta:
                lb_f32 = singles.tile([P, KB], fp32)
                nc.scalar.dma_start(
                    out=lb_f32, in_=lb_h[:].rearrange("(ko ki) -> ki ko", ki=P)
                )

            # ---- constants ----
            ident = singles.tile([P, P], bf16)
            make_identity(nc, ident)
            eps_t = singles.tile([P, 1], fp32)
            nc.vector.memset(eps_t, LN_EPS)
            ones_col = singles.tile([P, 1], fp32)
            nc.vector.memset(ones_col, 1.0)
            ones_row = singles.tile([1, P], fp32)
            nc.vector.memset(ones_row, 1.0)
            if apply_beta:
                lb_sb = singles.tile([P, KB], bf16)
                nc.vector.tensor_copy(out=lb_sb, in_=lb_f32)

            # ---- main-loop helpers ----
            f0_last = {}  # last DVE / ACT ops of super 0's front

            def front_super(j, x_sb=None):
                """Stats/normalize/transpose/copy for a super-tile."""
                if x_sb is None:
                    x_sb = issue_x(j)
                xts = []
                prev_cp = None
                for i in range(SUPER):
                    xt_ = x_sb[:, i, :]
                    st = stats_pool.tile([P, 2, 6], fp32, tag="st")
                    xr = xt_.rearrange("p (n f) -> p n f", f=512)
                    st0 = nc.vector.bn_stats(out=st[:, 0, :], in_=xr[:, 0, :])
                    if prev_cp is not None:
                        # tile0's xT copy unblocks the next MM block; don't
                        # let tile1's shallow stats sort ahead of it on DVE
                        add_dep_helper(
                            st0.ins, prev_cp.ins, sync=False,
                            reason="xT copy before next tile's stats on DVE",
                        )
                    nc.vector.bn_stats(out=st[:, 1, :], in_=xr[:, 1, :])
                    mv = stats_pool.tile([P, 2], fp32, tag="mv")
                    nc.vector.bn_aggr(out=mv, in_=st)
                    # s = 1/sqrt(var + eps)
                    s_t = stats_pool.tile([P, 1], fp32, tag="s")
                    sq0 = nc.scalar.activation(
                        out=s_t, in_=mv[:, 1:2], func=AF.Sqrt, bias=eps_t,
                        scale=1.0,
                    )
                    nc.vector.reciprocal(s_t, s_t)
                    # nb = -mu * s
                    nb = stats_pool.tile([P, 1], fp32, tag="nb")
                    nc.vector.tensor_scalar(
                        out=nb, in0=mv[:, 0:1], scalar1=s_t, scalar2=-1.0,
                        op0=OP.mult, op1=OP.mult,
                    )
                    # xn = (x - mu) * s, cast to bf16 (one fused ACT pass)
                    xn = xn_pool.tile([P, D], bf16)
                    norm0 = nc.scalar.activation(
                        out=xn, in_=xt_, func=AF.Identity, bias=nb, scale=s_t
                    )
                    # transpose to [d, tok] blocks for the PE contraction
                    ps_xt = ps_t_pool.tile([P, KB, P], bf16)
                    for k in range(KB):
                        nc.tensor.transpose(ps_xt[:, k, :], xn[:, ts(k, P)], ident)
                    xT = xt_pool.tile([P, KB, P], bf16)
                    cp0 = nc.vector.tensor_copy(out=xT, in_=ps_xt)
                    prev_cp = cp0
                    if j == 0 and i == 0:
                        # tile 0 only: tile 1 of super 0 lands after W, so
                        # gating the rowsums on it would serialize the head
                        f0_last["dve"] = cp0
                        f0_last["act"] = norm0
                    xts.append(xT)
                    # late supers must not preempt ternarize on DVE/ACT
                    if j >= TERN_DEP_FROM and tern_last_dve is not None:
                        add_dep_helper(
                            st0.ins, tern_last_dve.ins, sync=False,
                            reason="ternarize before late-super stats on DVE",
                        )
                        add_dep_helper(
                            sq0.ins, tern_last_act.ins, sync=False,
                            reason="ternarize before late-super sqrt on ACT",
                        )
                return xts

            wq = singles.tile([P, KB, U], bf16)  # holds 2*w_q
            beff128 = None
            bh128 = None
            tern_last_dve = None
            tern_last_act = None

            def back_super(j, xts):
                """Matmul sweep + epilogue + output DMA for a super-tile."""
                y_sb = y_pool.tile([P, SUPER, U], bf16)
                for i in range(SUPER):
                    # half-bank psum tiles: 6 rotating single-bank buffers
                    # give the ACT epilogue ~3 tiles of slack before the PE
                    # would stall on a psum reuse
                    ph = [
                        ps_y_pool.tile([P, 512], fp32, tag="ps_y", name=f"ph{h}")
                        for h in range(2)
                    ]
                    for k in range(KB):
                        for h in range(2):
                            nc.tensor.matmul(
                                ph[h],
                                lhsT=xts[i][:, k, :],
                                rhs=wq[:, k, ts(h, 512)],
                                start=(k == 0),
                                stop=(k == KB - 1),
                            )
                    # epilogue: y = psum * beta/2 (+ beta*b_eff), bf16 out
                    for h in range(2):
                        nc.scalar.mul(
                            out=y_sb[:, i, ts(h, 512)], in_=ph[h], mul=bh128
                        )
                    if apply_beta:
                        nc.vector.tensor_tensor(
                            y_sb[:, i, :], y_sb[:, i, :], beff128, OP.add
                        )
                if j == NTILES // SUPER - 1:
                    # final super: drain per-tile-half on both rings
                    for i in range(SUPER):
                        for h in range(2):
                            eng = nc.scalar if h == 0 else nc.sync
                            eng.dma_start(
                                out=y_view[:, j * SUPER + i, ts(h, 512)],
                                in_=y_sb[:, i, ts(h, 512)],
                            )
                else:
                    # y rides the scalar HWDGE ring (sync stays free for x)
                    nc.scalar.dma_start(
                        out=y_view[:, j * SUPER : (j + 1) * SUPER, :], in_=y_sb
                    )

            # ---- front for super 0 (ready long before W) ----
            pre = [front_super(0, x_sb=x_first)]

            # ---- weight prep: beta = mean|W| (pipelines with W chunks) ----
            # The static scheduler orders by DAG depth; these depth-1
            # reductions would head-of-line block super 0's deeper front ops
            # on both engine streams while waiting for W, so order them
            # explicitly after super 0's front.
            asum = singles.tile([P, KB], fp32)
            for k in range(KB):
                if k % 2 == 0:
                    rs = nc.vector.tensor_reduce(
                        out=asum[:, k : k + 1], in_=w_sb[:, k, :], axis=AX.X,
                        op=OP.add, apply_absolute_value=True,
                    )
                    add_dep_helper(
                        rs.ins, f0_last["dve"].ins, sync=False,
                        reason="super-0 front before |W| rowsums on DVE",
                    )
                else:
                    wabs_a = prep.tile([P, U], bf16, tag="absa")
                    rs = nc.scalar.activation(
                        out=wabs_a,
                        in_=w_sb[:, k, :],
                        func=AF.Abs,
                        accum_out=asum[:, k : k + 1],
                    )
                    add_dep_helper(
                        rs.ins, f0_last["act"].ins, sync=False,
                        reason="super-0 front before |W| rowsums on ACT",
                    )
            asum1 = singles.tile([P, 1], fp32)
            nc.vector.tensor_reduce(out=asum1, in_=asum, axis=AX.X, op=OP.add)

            # cross-partition total via ones-matmul -> scalar on partition 0
            ps_small = ps_y_pool.tile([P, 512], fp32, tag="ps_y", name="ps_small")
            nc.tensor.matmul(ps_small[0:1, 0:1], lhsT=ones_col, rhs=asum1)
            tot = singles.tile([1, 1], fp32)
            nc.vector.tensor_copy(out=tot, in_=ps_small[0:1, 0:1])
            # t1 = beta + EPS
            t1 = singles.tile([1, 1], fp32)
            nc.vector.tensor_scalar(
                out=t1, in0=tot, scalar1=1.0 / (D * U), scalar2=EPS,
                op0=OP.mult, op1=OP.add,
            )
            # pack3 = [c, -c, beta/2], c = 0.5*(beta+EPS)
            pack3 = singles.tile([1, 3], fp32)
            nc.vector.tensor_scalar(
                out=pack3[:, 0:1], in0=t1, scalar1=0.5, scalar2=None, op0=OP.mult
            )
            nc.vector.tensor_scalar(
                out=pack3[:, 1:2], in0=t1, scalar1=-0.5, scalar2=None, op0=OP.mult
            )
            nc.vector.tensor_scalar(
                out=pack3[:, 2:3], in0=tot, scalar1=0.5 / (D * U), scalar2=None,
                op0=OP.mult,
            )
            # broadcast to all 128 partitions (fresh ps_y slot)
            ps_small2 = ps_y_pool.tile([P, 512], fp32, tag="ps_y", name="ps_small2")
            nc.tensor.matmul(ps_small2[:, 0:3], lhsT=ones_row, rhs=pack3)
            rb128 = singles.tile([P, 3], fp32)
            nc.vector.tensor_copy(out=rb128, in_=ps_small2[:, 0:3])
            c128 = rb128[:, 0:1]
            negc128 = rb128[:, 1:2]
            bh128 = rb128[:, 2:3]  # beta/2 (wq is stored at 2x scale)

            # ---- fronts for supers 1-2 (no tern dep; fill the W window) ----
            pre += [front_super(j) for j in range(1, min(TERN_DEP_FROM, PRE_SUPERS))]

            # ---- ternarize, in MM consumption order (k ascending) ----
            # wq2 = sign(W-c) + sign(W+c) = 2*clip(round(W/(beta+eps)),-1,1)
            ps_beff = None
            if apply_beta:
                ps_beff = [
                    ps_y_pool.tile([P, 512], fp32, tag="ps_y", name=f"ps_beff{h}")
                    for h in range(2)
                ]
            for k in range(KB):
                p_t = prep.tile([P, U], bf16, tag="p")
                m_t = prep.tile([P, U], bf16, tag="m")
                if k % 2 == 0:
                    nc.vector.tensor_scalar(
                        out=p_t, in0=w_sb[:, k, :], scalar1=c128, scalar2=2.0,
                        op0=OP.is_gt, op1=OP.mult,
                    )
                    nc.vector.tensor_scalar(
                        out=m_t, in0=w_sb[:, k, :], scalar1=negc128, scalar2=2.0,
                        op0=OP.is_lt, op1=OP.mult,
                    )
                    tern_last_dve = nc.vector.tensor_tensor(
                        wq[:, k, :], p_t, m_t, OP.subtract
                    )
                else:
                    nc.scalar.activation(
                        out=p_t, in_=w_sb[:, k, :], func=AF.Sign, bias=negc128,
                        scale=1.0,
                    )
                    tern_last_act = nc.scalar.activation(
                        out=m_t, in_=w_sb[:, k, :], func=AF.Sign, bias=c128,
                        scale=1.0,
                    )
                    tern_last_dve = nc.vector.tensor_tensor(
                        wq[:, k, :], p_t, m_t, OP.add
                    )
                if apply_beta:
                    # b_eff[u] = sum_d ln_beta[d] * 2*wq[d, u]
                    for h in range(2):
                        nc.tensor.matmul(
                            ps_beff[h][0:1, :],
                            lhsT=lb_sb[:, k : k + 1],
                            rhs=wq[:, k, ts(h, 512)],
                            start=(k == 0),
                            stop=(k == KB - 1),
                        )
                if apply_gamma:
                    tern_last_dve = nc.vector.tensor_scalar(
                        out=wq[:, k, :], in0=wq[:, k, :],
                        scalar1=g_sb[:, k : k + 1], scalar2=None, op0=OP.mult,
                    )

            if apply_beta:
                beff = singles.tile([1, U], fp32)
                # scale by beta/2 now so the epilogue is a plain add
                for h in range(2):
                    nc.vector.tensor_scalar(
                        out=beff[:, ts(h, 512)], in0=ps_beff[h][0:1, :],
                        scalar1=bh128[0:1, 0:1], scalar2=None, op0=OP.mult,
                    )
                beff128_f = singles.tile([P, U], fp32)
                for h in range(2):
                    ps_b2 = ps_y_pool.tile([P, 512], fp32, tag="ps_y")
                    nc.tensor.matmul(ps_b2, lhsT=ones_row, rhs=beff[:, ts(h, 512)])
                    nc.vector.tensor_copy(out=beff128_f[:, ts(h, 512)], in_=ps_b2)
                beff128 = beff128_f

            # ---- fronts for supers 3..5 (dep-ordered after ternarize) ----
            pre += [front_super(j) for j in range(TERN_DEP_FROM, PRE_SUPERS)]

            # ---- main loop ----
            NJ = NTILES // SUPER
            for j in range(PRE_SUPERS):
                back_super(j, pre[j])
            for j in range(PRE_SUPERS, NJ):
                back_super(j, front_super(j))

    nc.compile()
    return nc


def _get_nc(apply_gamma: bool, apply_beta: bool):
    key = (apply_gamma, apply_beta)
    if key not in _NC_CACHE:
        _NC_CACHE[key] = _build(apply_gamma, apply_beta)
    return _NC_CACHE[key]


def _make_in_maps(x, w, g, lb, apply_gamma, apply_beta):
    xf = np.ascontiguousarray(x.reshape(B * S, D))
    in_maps = []
    for c in range(N_CORES):
        m = {
            "x": np.ascontiguousarray(xf[c * TOK : (c + 1) * TOK]),
            "weight": w,
        }
        if apply_gamma:
            m["ln_gamma"] = g
        if apply_beta:
            m["ln_beta"] = lb
        in_maps.append(m)
    return in_maps


def run(inputs, trace=False, tmpdir=None):
    """Shard, run on 8 cores, gather. Returns (y, BassKernelResults)."""
    from concourse.bass_utils import run_bass_kernel_spmd

    x = np.asarray(inputs["x"], dtype=np.float32)
    w = np.ascontiguousarray(np.asarray(inputs["weight"], dtype=np.float32))
    g = np.ascontiguousarray(np.asarray(inputs["ln_gamma"], dtype=np.float32))
    lb = np.ascontiguousarray(np.asarray(inputs["ln_beta"], dtype=np.float32))
    apply_gamma = not bool(np.all(g == 1.0))
    apply_beta = not bool(np.all(lb == 0.0))

    nc = _get_nc(apply_gamma, apply_beta)
    in_maps = _make_in_maps(x, w, g, lb, apply_gamma, apply_beta)
    res = run_bass_kernel_spmd(
        nc, in_maps, core_ids=list(range(N_CORES)), trace=trace, tmpdir=tmpdir
    )
    y = np.concatenate(
        [np.asarray(r["y"], dtype=np.float32) for r in res.results], axis=0
    )
    return y.reshape(B, S, U), res


def kernel(**inputs) -> np.ndarray:
    y, _ = run(inputs, trace=False)
    return y
